# revision 20
# baseline (speedup 1.0000x reference)
"""Trainium2 Bass kernel for nn_MultiHeadAttention_56375740727430.

Causal multi-head attention, B=2 S=2048 D=1024 H=16 KS=64, followed by an
output projection `heads @ kernel`.

Sharding: pure data/head parallel over 8 cores — core c handles batch c//4
and 4 heads (c%4)*4 ... +4.  Each core computes Q^T/K^T (head-pair-stacked,
transposed layout), V (natural layout, with an appended ones-column so the
softmax denominator Z falls out of the attention matmul), causal scores ->
exp -> (P@V | Z) -> per-head output projection, all unnormalized.  The host
divides by Z, sums head contributions and batch-partials, and transposes.

Matmul operands are bf16 (1 col/cycle on the PE); accumulation, scores (exp
input) and Z stay fp32.

Schedule:
- inputs are host-pre-shuffled to partition-major layouts and DMA'd in
  d-tile pairs (large descriptors, ~full HBM bandwidth);
- projections emitted t-major (one PSUM accumulator per output chunk, all 8
  banks) so the PE starts as soon as the first x d-tile pair lands and
  overlaps the input DMA stream;
- attention inner loop software-pipelined: scores(lt+1) is emitted before
  PV(lt), so the PE never sits behind the exp on the critical path and PV
  weight loads prefetch during the preceding score pair; each block's
  output projection + output DMA are deferred into the next block;
- the two head pairs' (pr, query-block) units are interleaved so the
  PE-heavy pr0 blocks overlap the exp-bound pr1 stretch; qk1 filler chunks
  hide inside pr0's blocks (2 per block, matching the PSUM ring);
- the exp table is preloaded during the projection phase; the two heads of
  the output projection run as a block-diagonal pair via tile_position;
  outputs are DMA'd per (pair, query block).
"""

import sys

sys.path.insert(0, "/opt/trn_rl_repo")

from contextlib import ExitStack

import ml_dtypes
import numpy as np

import concourse.bass as bass
import concourse.bacc as bacc
import concourse.mybir as mybir
import concourse.tile as tile

B, S, D = 2, 2048, 1024
H, KS = 16, 64

P = 128            # partitions
NCORES = 8
CORES_PER_B = NCORES // B          # 4
NH = H // CORES_PER_B              # heads per core = 4
NW = NH * KS                       # per-core projection width = 256
DT = D // P                        # d-tiles = 8
ST = S // P                        # s/l-tiles = 16
IB = 512                           # query block
NIB = S // IB                      # 4
LPB = IB // P                      # l-tiles per query block = 4
NWARM = 6                          # PE clock-ramp matmuls

F32 = mybir.dt.float32
BF16 = mybir.dt.bfloat16
NP_BF16 = ml_dtypes.bfloat16
EXP = mybir.ActivationFunctionType.Exp


def build_nc():
    mm_dt = BF16
    nc = bacc.Bacc()

    # pre-shuffled on host: partition-major so each partition reads one
    # contiguous run per DMA (fewer, larger descriptors)
    xT = nc.declare_dram_parameter("xT", [P, DT, S], mm_dt, isOutput=False)
    wq = nc.declare_dram_parameter("wq", [P, DT, NW], mm_dt, isOutput=False)
    wk = nc.declare_dram_parameter("wk", [P, DT, NW], mm_dt, isOutput=False)
    wv = nc.declare_dram_parameter("wv", [P, DT, NW], mm_dt, isOutput=False)
    # pair layout: partition = hh*KS + k, dims = [head pair, j]
    wkern = nc.declare_dram_parameter("wkern", [P, NH // 2, KS], mm_dt, isOutput=False)
    masks = nc.declare_dram_parameter("masks", [P, 2, P], mm_dt, isOutput=False)
    outT = nc.declare_dram_parameter("outT", [KS, NIB, NH, IB], mm_dt, isOutput=True)
    z = nc.declare_dram_parameter("z", [NH, S], F32, isOutput=True)

    with tile.TileContext(nc) as tc, ExitStack() as ctx:
        const_pool = ctx.enter_context(tc.tile_pool(name="const", bufs=1))
        qkv_pool = ctx.enter_context(tc.tile_pool(name="qkv", bufs=1))
        out_pool = ctx.enter_context(tc.tile_pool(name="outp", bufs=1))
        xw_pool = ctx.enter_context(tc.tile_pool(name="xw", bufs=1))
        pexp_pool = ctx.enter_context(tc.tile_pool(name="pexp", bufs=6))
        osb_pool = ctx.enter_context(tc.tile_pool(name="osb", bufs=4))

        warm_in = const_pool.tile([P, IB], mm_dt)
        nc.gpsimd.memset(warm_in[:], 0.0)
        # touch Exp during the projection phase so the ~1.3us ACT_TABLE_LOAD
        # doesn't land on the first real softmax tile
        warm_exp = const_pool.tile([P, 1], F32)
        nc.scalar.activation(warm_exp[:], warm_in[:, 0:1], EXP)

        qt_sb = [
            qkv_pool.tile([P, S], mm_dt, tag=f"qt{i}", name=f"qt{i}") for i in range(2)
        ]
        kt_sb = [
            qkv_pool.tile([P, S], mm_dt, tag=f"kt{i}", name=f"kt{i}") for i in range(2)
        ]
        v_sb = qkv_pool.tile([P, ST, NH, KS + 1], mm_dt, tag="v")
        nc.gpsimd.memset(v_sb[:, :, :, KS], 1.0)
        outT_sb = out_pool.tile([KS, NH, S], mm_dt)

        # input DMA: small consts first, then per-d-tile weight + x slices in
        # t order so the t-major projection waves start as soon as possible
        wkern_sb = const_pool.tile([P, NH // 2, KS], mm_dt)
        nc.sync.dma_start(wkern_sb[:], wkern[:])
        mask_sb = const_pool.tile([P, 2, P], mm_dt)
        nc.sync.dma_start(mask_sb[:], masks[:])

        w_sb = {}
        for name, wh in (("q", wq), ("k", wk), ("v", wv)):
            w_sb[name] = xw_pool.tile(
                [P, DT, NW], mm_dt, tag=f"w{name}", name=f"w{name}"
            )
        xT_sb = xw_pool.tile([P, DT, S], mm_dt, tag="xT")
        for tp in range(DT // 2):
            s2 = slice(2 * tp, 2 * tp + 2)
            for name, wh in (("q", wq), ("k", wk), ("v", wv)):
                nc.sync.dma_start(w_sb[name][:, s2, :], wh[:, s2, :])
            nc.sync.dma_start(xT_sb[:, s2, :], xT[:, s2, :])

        # ---- projection phase: t-major waves over 8 PSUM accumulators.
        # One pool for warmup + Q/K waves + V waves so V's accumulators
        # pipeline into slots as the Q/K copies retire (no barrier).
        with tc.tile_pool(name="pproj", bufs=8, space=bass.MemorySpace.PSUM) as pproj:
            for _ in range(NWARM):
                w_ps = pproj.tile([P, IB], F32, tag="of", name="w_ps")
                nc.tensor.matmul(
                    w_ps[:], warm_in[:, 0:P], warm_in[:], start=True, stop=True
                )
            chunks = [(wn, ic) for wn in ("q", "k") for ic in range(NIB)]
            ps = {
                c: pproj.tile([P, IB], F32, tag="of", name=f"qk0_{c[0]}{c[1]}")
                for c in chunks
            }
            for t in range(DT):
                for wn, ic in chunks:
                    nc.tensor.matmul(
                        ps[(wn, ic)][:],
                        w_sb[wn][:, t, 0:P],
                        xT_sb[:, t, ic * IB : (ic + 1) * IB],
                        start=(t == 0),
                        stop=(t == DT - 1),
                    )
                    if t == DT - 1:
                        dst = qt_sb if wn == "q" else kt_sb
                        nc.vector.tensor_copy(
                            dst[0][:, ic * IB : (ic + 1) * IB], ps[(wn, ic)][:]
                        )
            for g in range(2):
                sts = range(g * 8, (g + 1) * 8)
                vs = {
                    st: pproj.tile([P, NW], F32, tag="of", name=f"v{st}")
                    for st in sts
                }
                for t in range(DT):
                    for st in sts:
                        nc.tensor.matmul(
                            vs[st][:],
                            xT_sb[:, t, st * P : (st + 1) * P],
                            w_sb["v"][:, t, :],
                            start=(t == 0),
                            stop=(t == DT - 1),
                        )
                        if t == DT - 1:
                            nc.vector.tensor_copy(
                                v_sb[:, st, :, 0:KS],
                                vs[st][:].rearrange("p (h k) -> p h k", k=KS),
                            )

        pst = ctx.enter_context(
            tc.tile_pool(name="pst", bufs=2, space=bass.MemorySpace.PSUM)
        )
        po = ctx.enter_context(
            tc.tile_pool(name="po", bufs=4, space=bass.MemorySpace.PSUM)
        )

        def qk1_chunk(wn, ic):
            cps = po.tile([P, IB], F32, tag="of", name=f"qk1_{wn}{ic}")
            for t in range(DT):
                nc.tensor.matmul(
                    cps[:],
                    w_sb[wn][:, t, P : 2 * P],
                    xT_sb[:, t, ic * IB : (ic + 1) * IB],
                    start=(t == 0),
                    stop=(t == DT - 1),
                )
            dst = qt_sb if wn == "q" else kt_sb
            nc.vector.tensor_copy(dst[1][:, ic * IB : (ic + 1) * IB], cps[:])

        pending_end = [None]

        def attention(blocks, fillers):
            # causal attention + output projection, one (head pair, query
            # block) at a time; scores row-packed via tile_position so both
            # heads' K=64 matmuls share the PE array.  Inner loop is
            # software-pipelined: PV(lt) is emitted after scores(lt+1) so
            # the PE isn't gated on exp(lt), and each block's output
            # projection is deferred into the next block.
            for pr, ib in blocks:
                nl = (ib + 1) * LPB
                o_ps = [
                    po.tile([KS + 1, IB], F32, tag="of", name=f"o{pr}_{ib}_{hh}")
                    for hh in range(2)
                ]

                def emit_pv(lt, pe, off):
                    for hh in range(2):
                        nc.tensor.matmul(
                            o_ps[hh][:, off:IB],
                            v_sb[:, lt, 2 * pr + hh, :],
                            pe[:, hh, off:IB],
                            start=(lt == 0),
                            stop=(lt == nl - 1),
                        )

                pending = None
                for lt in range(nl):
                    # causal: columns [0, off) of this i-block are fully
                    # masked for key tile lt; compute only the suffix
                    off = max(0, (lt - ib * LPB)) * P
                    st_ps = pst.tile([P, 2, IB], F32, tag="st", name="st")
                    for hh in range(2):
                        nc.tensor.matmul(
                            st_ps[:, hh, off:IB],
                            kt_sb[pr][hh * KS : (hh + 1) * KS, lt * P : (lt + 1) * P],
                            qt_sb[pr][
                                hh * KS : (hh + 1) * KS,
                                ib * IB + off : (ib + 1) * IB,
                            ],
                            start=True,
                            stop=True,
                            tile_position=(hh * KS, 0),
                        )
                    pe = pexp_pool.tile([P, 2, IB], BF16, tag="pe", name="pe")
                    nc.scalar.activation(
                        pe[:, :, off:IB], st_ps[:, :, off:IB], EXP, scale=0.125
                    )
                    if lt >= ib * LPB:  # diagonal 128-block -> triangular mask
                        nc.vector.tensor_mul(
                            pe[:, :, off : off + P],
                            pe[:, :, off : off + P],
                            mask_sb[:],
                        )
                    if lt == 1 and pending_end[0] is not None:
                        pending_end[0]()
                        pending_end[0] = None
                    if pending is not None:
                        emit_pv(*pending)
                    if lt in (2, 3) and fillers:
                        fillers.pop(0)()
                    pending = (lt, pe, off)
                emit_pv(*pending)
                pending_end[0] = (
                    lambda pr=pr, ib=ib, o_ps=o_ps: emit_ib_end(pr, ib, o_ps)
                )

        def emit_ib_end(pr, ib, o_ps):
            # both heads' output projections as one block-diagonal pair
            o_bf = osb_pool.tile([P, IB], BF16, tag="o_bf", name="o_bf")
            for hh in range(2):
                nc.vector.tensor_copy(
                    o_bf[hh * KS : (hh + 1) * KS, :], o_ps[hh][0:KS, :]
                )
            for hh in range(2):
                z_sb = osb_pool.tile([1, IB], F32, tag="z_sb", name="z_sb")
                nc.vector.tensor_copy(
                    z_sb[0:1, :], o_ps[hh][KS : KS + 1, :]
                )
                nc.sync.dma_start(
                    z[2 * pr + hh, ib * IB : (ib + 1) * IB], z_sb[0:1, :]
                )
            f_ps = po.tile([P, IB], F32, tag="of", name="f_ps")
            for hh in range(2):
                nc.tensor.matmul(
                    f_ps[hh * KS : (hh + 1) * KS, :],
                    wkern_sb[hh * KS : (hh + 1) * KS, pr, :],
                    o_bf[hh * KS : (hh + 1) * KS, :],
                    start=True,
                    stop=True,
                    tile_position=(hh * KS, hh * KS),
                )
            for hh in range(2):
                h = 2 * pr + hh
                nc.vector.tensor_copy(
                    outT_sb[:, h, ib * IB : (ib + 1) * IB],
                    f_ps[hh * KS : (hh + 1) * KS, :],
                )
            nc.sync.dma_start(
                outT[:, ib, 2 * pr : 2 * pr + 2, :],
                outT_sb[:, 2 * pr : 2 * pr + 2, ib * IB : (ib + 1) * IB],
            )

        fillers = [
            (lambda wn=wn, ic=ic: qk1_chunk(wn, ic))
            for ic in range(NIB)
            for wn in ("q", "k")
        ]
        # pr0's first three blocks hide the qk1 filler chunks; pr1 blocks
        # then interleave with pr0's biggest block so the exp-bound and
        # PE-bound stretches overlap instead of running as separate phases.
        attention([(0, 0), (0, 1), (0, 2)], fillers)
        # leftover chunks ride in (1,0)'s slots: they write the ic=3 quarter
        # of qt1/kt1, which is first read much later, by block (1,3)
        attention([(1, 0), (0, 3), (1, 1), (1, 2), (1, 3)], fillers)
        pending_end[0]()  # final query block's projection + output DMA
        pending_end[0] = None

    nc.compile()
    return nc


def make_masks():
    # triangular [P, P] stacked for both heads of a pair: keep j >= p
    j = np.arange(P)[None, :]
    p = np.arange(P)[:, None]
    m = (j >= p).astype(NP_BF16)
    return np.stack([m, m], axis=1)  # [P, 2, P]


def make_in_maps(inputs):
    x = np.asarray(inputs["x"], np.float32)
    Wq = np.asarray(inputs["Wq"], np.float32)
    Wk = np.asarray(inputs["Wk"], np.float32)
    Wv = np.asarray(inputs["Wv"], np.float32)
    kern = np.asarray(inputs["kernel"], np.float32)

    masks = make_masks()
    kern3 = kern.reshape(KS, H, KS)  # [k, h, j]
    in_maps = []
    for c in range(NCORES):
        b, hs = c // CORES_PER_B, (c % CORES_PER_B) * NH
        # wkern pair layout: [hh*KS + k, pair, j] for heads h = hs + 2*pair + hh
        kern_c = kern3[:, hs : hs + NH, :]  # [k, h, j]
        wkern_pair = np.zeros((P, NH // 2, KS), np.float32)
        for pair in range(NH // 2):
            for hh in range(2):
                wkern_pair[hh * KS : (hh + 1) * KS, pair, :] = kern_c[
                    :, 2 * pair + hh, :
                ]
        in_maps.append(
            {
                "xT": x[b].T.reshape(DT, P, S).transpose(1, 0, 2)
                .astype(NP_BF16),
                "wq": Wq[:, :, hs : hs + NH].transpose(0, 2, 1).reshape(DT, P, NW)
                .transpose(1, 0, 2).astype(NP_BF16),
                "wk": Wk[:, :, hs : hs + NH].transpose(0, 2, 1).reshape(DT, P, NW)
                .transpose(1, 0, 2).astype(NP_BF16),
                "wv": Wv[:, :, hs : hs + NH].transpose(0, 2, 1).reshape(DT, P, NW)
                .transpose(1, 0, 2).astype(NP_BF16),
                "wkern": wkern_pair.astype(NP_BF16),
                "masks": masks,
            }
        )
    return in_maps


def gather_output(results):
    out = np.zeros((B, S, KS), np.float32)
    for c in range(NCORES):
        b = c // CORES_PER_B
        oT = (
            np.asarray(results[c]["outT"], np.float32)  # [KS, NIB, NH, IB]
            .transpose(2, 0, 1, 3)
            .reshape(NH, KS, S)
        )
        zz = np.asarray(results[c]["z"], np.float32)     # [NH, S]
        out[b] += (oT / zz[:, None, :]).sum(axis=0).T
    return out


_NC_CACHE = {}


def get_nc():
    if "nc" not in _NC_CACHE:
        _NC_CACHE["nc"] = build_nc()
    return _NC_CACHE["nc"]


def run_hw(inputs, trace=False, **kw):
    from concourse.bass_utils import run_bass_kernel_spmd

    nc = get_nc()
    in_maps = make_in_maps(inputs)
    res = run_bass_kernel_spmd(
        nc, in_maps, list(range(NCORES)), trace=trace, **kw
    )
    return gather_output(res.results), res


def kernel(**inputs) -> np.ndarray:
    out, _ = run_hw(inputs, trace=False)
    return out


# revision 21
# speedup vs baseline: 1.0083x; 1.0083x over previous
"""Trainium2 Bass kernel for nn_MultiHeadAttention_56375740727430.

Causal multi-head attention, B=2 S=2048 D=1024 H=16 KS=64, followed by an
output projection `heads @ kernel`.

Sharding: pure data/head parallel over 8 cores — core c handles batch c//4
and 4 heads (c%4)*4 ... +4.  Each core computes Q^T/K^T (head-pair-stacked,
transposed layout), V (natural layout, with an appended ones-column so the
softmax denominator Z falls out of the attention matmul), causal scores ->
exp -> (P@V | Z) -> per-head output projection, all unnormalized.  The host
divides by Z, sums head contributions and batch-partials, and transposes.

Matmul operands are bf16 (1 col/cycle on the PE); accumulation, scores (exp
input) and Z stay fp32.

Schedule:
- inputs are host-pre-shuffled to partition-major layouts and DMA'd in
  d-tile pairs (large descriptors, ~full HBM bandwidth);
- projections emitted t-major (one PSUM accumulator per output chunk, all 8
  banks) so the PE starts as soon as the first x d-tile pair lands and
  overlaps the input DMA stream;
- attention inner loop software-pipelined: scores(lt+1) is emitted before
  PV(lt), so the PE never sits behind the exp on the critical path and PV
  weight loads prefetch during the preceding score pair; each block's
  output projection + output DMA are deferred into the next block;
- the two head pairs' (pr, query-block) units are interleaved so the
  PE-heavy pr0 blocks overlap the exp-bound pr1 stretch; qk1 filler chunks
  hide inside pr0's blocks (2 per block, matching the PSUM ring);
- the exp table is preloaded during the projection phase; the two heads of
  the output projection run as a block-diagonal pair via tile_position;
  outputs are DMA'd per (pair, query block).
"""

import sys

sys.path.insert(0, "/opt/trn_rl_repo")

from contextlib import ExitStack

import ml_dtypes
import numpy as np

import concourse.bass as bass
import concourse.bacc as bacc
import concourse.mybir as mybir
import concourse.tile as tile

B, S, D = 2, 2048, 1024
H, KS = 16, 64

P = 128            # partitions
NCORES = 8
CORES_PER_B = NCORES // B          # 4
NH = H // CORES_PER_B              # heads per core = 4
NW = NH * KS                       # per-core projection width = 256
DT = D // P                        # d-tiles = 8
ST = S // P                        # s/l-tiles = 16
IB = 512                           # query block
NIB = S // IB                      # 4
LPB = IB // P                      # l-tiles per query block = 4
NWARM = 16                         # PE clock-ramp matmuls (bridge the x-DMA wait)

F32 = mybir.dt.float32
BF16 = mybir.dt.bfloat16
NP_BF16 = ml_dtypes.bfloat16
EXP = mybir.ActivationFunctionType.Exp


def build_nc():
    mm_dt = BF16
    nc = bacc.Bacc()

    # pre-shuffled on host: partition-major so each partition reads one
    # contiguous run per DMA (fewer, larger descriptors)
    xT = nc.declare_dram_parameter("xT", [P, DT, S], mm_dt, isOutput=False)
    wq = nc.declare_dram_parameter("wq", [P, DT, NW], mm_dt, isOutput=False)
    wk = nc.declare_dram_parameter("wk", [P, DT, NW], mm_dt, isOutput=False)
    wv = nc.declare_dram_parameter("wv", [P, DT, NW], mm_dt, isOutput=False)
    # pair layout: partition = hh*KS + k, dims = [head pair, j]
    wkern = nc.declare_dram_parameter("wkern", [P, NH // 2, KS], mm_dt, isOutput=False)
    masks = nc.declare_dram_parameter("masks", [P, 2, P], mm_dt, isOutput=False)
    outT = nc.declare_dram_parameter("outT", [KS, NIB, NH, IB], mm_dt, isOutput=True)
    z = nc.declare_dram_parameter("z", [NH, S], F32, isOutput=True)

    with tile.TileContext(nc) as tc, ExitStack() as ctx:
        const_pool = ctx.enter_context(tc.tile_pool(name="const", bufs=1))
        qkv_pool = ctx.enter_context(tc.tile_pool(name="qkv", bufs=1))
        out_pool = ctx.enter_context(tc.tile_pool(name="outp", bufs=1))
        xw_pool = ctx.enter_context(tc.tile_pool(name="xw", bufs=1))
        pexp_pool = ctx.enter_context(tc.tile_pool(name="pexp", bufs=6))
        osb_pool = ctx.enter_context(tc.tile_pool(name="osb", bufs=4))

        warm_in = const_pool.tile([P, IB], mm_dt)
        nc.gpsimd.memset(warm_in[:], 0.0)
        # touch Exp during the projection phase so the ~1.3us ACT_TABLE_LOAD
        # doesn't land on the first real softmax tile
        warm_exp = const_pool.tile([P, 1], F32)
        nc.scalar.activation(warm_exp[:], warm_in[:, 0:1], EXP)

        qt_sb = [
            qkv_pool.tile([P, S], mm_dt, tag=f"qt{i}", name=f"qt{i}") for i in range(2)
        ]
        kt_sb = [
            qkv_pool.tile([P, S], mm_dt, tag=f"kt{i}", name=f"kt{i}") for i in range(2)
        ]
        v_sb = qkv_pool.tile([P, ST, NH, KS + 1], mm_dt, tag="v")
        nc.gpsimd.memset(v_sb[:, :, :, KS], 1.0)
        outT_sb = out_pool.tile([KS, NH, S], mm_dt)

        # input DMA: small consts first, then per-d-tile weight + x slices in
        # t order so the t-major projection waves start as soon as possible
        wkern_sb = const_pool.tile([P, NH // 2, KS], mm_dt)
        nc.sync.dma_start(wkern_sb[:], wkern[:])
        mask_sb = const_pool.tile([P, 2, P], mm_dt)
        nc.sync.dma_start(mask_sb[:], masks[:])

        w_sb = {}
        for name, wh in (("q", wq), ("k", wk), ("v", wv)):
            w_sb[name] = xw_pool.tile(
                [P, DT, NW], mm_dt, tag=f"w{name}", name=f"w{name}"
            )
        xT_sb = xw_pool.tile([P, DT, S], mm_dt, tag="xT")
        for tp in range(DT // 2):
            s2 = slice(2 * tp, 2 * tp + 2)
            for name, wh in (("q", wq), ("k", wk)):
                nc.sync.dma_start(w_sb[name][:, s2, :], wh[:, s2, :])
            nc.sync.dma_start(xT_sb[:, s2, :], xT[:, s2, :])
        nc.sync.dma_start(w_sb["v"][:], wv[:])  # not needed until the V waves

        # ---- projection phase: t-major waves over 8 PSUM accumulators.
        # One pool for warmup + Q/K waves + V waves so V's accumulators
        # pipeline into slots as the Q/K copies retire (no barrier).
        with tc.tile_pool(name="pproj", bufs=8, space=bass.MemorySpace.PSUM) as pproj:
            for _ in range(NWARM):
                w_ps = pproj.tile([P, IB], F32, tag="of", name="w_ps")
                nc.tensor.matmul(
                    w_ps[:], warm_in[:, 0:P], warm_in[:], start=True, stop=True
                )
            chunks = [(wn, ic) for wn in ("q", "k") for ic in range(NIB)]
            ps = {
                c: pproj.tile([P, IB], F32, tag="of", name=f"qk0_{c[0]}{c[1]}")
                for c in chunks
            }
            for t in range(DT):
                for wn, ic in chunks:
                    nc.tensor.matmul(
                        ps[(wn, ic)][:],
                        w_sb[wn][:, t, 0:P],
                        xT_sb[:, t, ic * IB : (ic + 1) * IB],
                        start=(t == 0),
                        stop=(t == DT - 1),
                    )
                    if t == DT - 1:
                        dst = qt_sb if wn == "q" else kt_sb
                        nc.vector.tensor_copy(
                            dst[0][:, ic * IB : (ic + 1) * IB], ps[(wn, ic)][:]
                        )
            for g in range(2):
                sts = range(g * 8, (g + 1) * 8)
                vs = {
                    st: pproj.tile([P, NW], F32, tag="of", name=f"v{st}")
                    for st in sts
                }
                for t in range(DT):
                    for st in sts:
                        nc.tensor.matmul(
                            vs[st][:],
                            xT_sb[:, t, st * P : (st + 1) * P],
                            w_sb["v"][:, t, :],
                            start=(t == 0),
                            stop=(t == DT - 1),
                        )
                        if t == DT - 1:
                            nc.vector.tensor_copy(
                                v_sb[:, st, :, 0:KS],
                                vs[st][:].rearrange("p (h k) -> p h k", k=KS),
                            )

        pst = ctx.enter_context(
            tc.tile_pool(name="pst", bufs=2, space=bass.MemorySpace.PSUM)
        )
        po = ctx.enter_context(
            tc.tile_pool(name="po", bufs=4, space=bass.MemorySpace.PSUM)
        )

        def qk1_chunk(wn, ic):
            cps = po.tile([P, IB], F32, tag="of", name=f"qk1_{wn}{ic}")
            for t in range(DT):
                nc.tensor.matmul(
                    cps[:],
                    w_sb[wn][:, t, P : 2 * P],
                    xT_sb[:, t, ic * IB : (ic + 1) * IB],
                    start=(t == 0),
                    stop=(t == DT - 1),
                )
            dst = qt_sb if wn == "q" else kt_sb
            nc.vector.tensor_copy(dst[1][:, ic * IB : (ic + 1) * IB], cps[:])

        pending_end = [None]

        def attention(blocks, fillers):
            # causal attention + output projection, one (head pair, query
            # block) at a time; scores row-packed via tile_position so both
            # heads' K=64 matmuls share the PE array.  Inner loop is
            # software-pipelined: PV(lt) is emitted after scores(lt+1) so
            # the PE isn't gated on exp(lt), and each block's output
            # projection is deferred into the next block.
            for pr, ib in blocks:
                nl = (ib + 1) * LPB
                o_ps = [
                    po.tile([KS + 1, IB], F32, tag="of", name=f"o{pr}_{ib}_{hh}")
                    for hh in range(2)
                ]

                def emit_pv(lt, pe, off):
                    for hh in range(2):
                        nc.tensor.matmul(
                            o_ps[hh][:, off:IB],
                            v_sb[:, lt, 2 * pr + hh, :],
                            pe[:, hh, off:IB],
                            start=(lt == 0),
                            stop=(lt == nl - 1),
                        )

                pending = None
                for lt in range(nl):
                    # causal: columns [0, off) of this i-block are fully
                    # masked for key tile lt; compute only the suffix
                    off = max(0, (lt - ib * LPB)) * P
                    st_ps = pst.tile([P, 2, IB], F32, tag="st", name="st")
                    for hh in range(2):
                        nc.tensor.matmul(
                            st_ps[:, hh, off:IB],
                            kt_sb[pr][hh * KS : (hh + 1) * KS, lt * P : (lt + 1) * P],
                            qt_sb[pr][
                                hh * KS : (hh + 1) * KS,
                                ib * IB + off : (ib + 1) * IB,
                            ],
                            start=True,
                            stop=True,
                            tile_position=(hh * KS, 0),
                        )
                    pe = pexp_pool.tile([P, 2, IB], BF16, tag="pe", name="pe")
                    nc.scalar.activation(
                        pe[:, :, off:IB], st_ps[:, :, off:IB], EXP, scale=0.125
                    )
                    if lt >= ib * LPB:  # diagonal 128-block -> triangular mask
                        nc.vector.tensor_mul(
                            pe[:, :, off : off + P],
                            pe[:, :, off : off + P],
                            mask_sb[:],
                        )
                    if lt == 1 and pending_end[0] is not None:
                        pending_end[0]()
                        pending_end[0] = None
                    if pending is not None:
                        emit_pv(*pending)
                    if lt in (2, 3) and fillers:
                        fillers.pop(0)()
                    pending = (lt, pe, off)
                emit_pv(*pending)
                pending_end[0] = (
                    lambda pr=pr, ib=ib, o_ps=o_ps: emit_ib_end(pr, ib, o_ps)
                )

        def emit_ib_end(pr, ib, o_ps):
            # both heads' output projections as one block-diagonal pair
            o_bf = osb_pool.tile([P, IB], BF16, tag="o_bf", name="o_bf")
            for hh in range(2):
                nc.vector.tensor_copy(
                    o_bf[hh * KS : (hh + 1) * KS, :], o_ps[hh][0:KS, :]
                )
            for hh in range(2):
                z_sb = osb_pool.tile([1, IB], F32, tag="z_sb", name="z_sb")
                nc.vector.tensor_copy(
                    z_sb[0:1, :], o_ps[hh][KS : KS + 1, :]
                )
                nc.sync.dma_start(
                    z[2 * pr + hh, ib * IB : (ib + 1) * IB], z_sb[0:1, :]
                )
            f_ps = po.tile([P, IB], F32, tag="of", name="f_ps")
            for hh in range(2):
                nc.tensor.matmul(
                    f_ps[hh * KS : (hh + 1) * KS, :],
                    wkern_sb[hh * KS : (hh + 1) * KS, pr, :],
                    o_bf[hh * KS : (hh + 1) * KS, :],
                    start=True,
                    stop=True,
                    tile_position=(hh * KS, hh * KS),
                )
            for hh in range(2):
                h = 2 * pr + hh
                nc.vector.tensor_copy(
                    outT_sb[:, h, ib * IB : (ib + 1) * IB],
                    f_ps[hh * KS : (hh + 1) * KS, :],
                )
            nc.sync.dma_start(
                outT[:, ib, 2 * pr : 2 * pr + 2, :],
                outT_sb[:, 2 * pr : 2 * pr + 2, ib * IB : (ib + 1) * IB],
            )

        fillers = [
            (lambda wn=wn, ic=ic: qk1_chunk(wn, ic))
            for ic in range(NIB)
            for wn in ("q", "k")
        ]
        # pr0's first three blocks hide the qk1 filler chunks; pr1 blocks
        # then interleave with pr0's biggest block so the exp-bound and
        # PE-bound stretches overlap instead of running as separate phases.
        attention([(0, 0), (0, 1), (0, 2)], fillers)
        # leftover chunks ride in (1,0)'s slots: they write the ic=3 quarter
        # of qt1/kt1, which is first read much later, by block (1,3)
        attention([(1, 0), (0, 3), (1, 1), (1, 2), (1, 3)], fillers)
        pending_end[0]()  # final query block's projection + output DMA
        pending_end[0] = None

    nc.compile()
    return nc


def make_masks():
    # triangular [P, P] stacked for both heads of a pair: keep j >= p
    j = np.arange(P)[None, :]
    p = np.arange(P)[:, None]
    m = (j >= p).astype(NP_BF16)
    return np.stack([m, m], axis=1)  # [P, 2, P]


def make_in_maps(inputs):
    x = np.asarray(inputs["x"], np.float32)
    Wq = np.asarray(inputs["Wq"], np.float32)
    Wk = np.asarray(inputs["Wk"], np.float32)
    Wv = np.asarray(inputs["Wv"], np.float32)
    kern = np.asarray(inputs["kernel"], np.float32)

    masks = make_masks()
    kern3 = kern.reshape(KS, H, KS)  # [k, h, j]
    in_maps = []
    for c in range(NCORES):
        b, hs = c // CORES_PER_B, (c % CORES_PER_B) * NH
        # wkern pair layout: [hh*KS + k, pair, j] for heads h = hs + 2*pair + hh
        kern_c = kern3[:, hs : hs + NH, :]  # [k, h, j]
        wkern_pair = np.zeros((P, NH // 2, KS), np.float32)
        for pair in range(NH // 2):
            for hh in range(2):
                wkern_pair[hh * KS : (hh + 1) * KS, pair, :] = kern_c[
                    :, 2 * pair + hh, :
                ]
        in_maps.append(
            {
                "xT": x[b].T.reshape(DT, P, S).transpose(1, 0, 2)
                .astype(NP_BF16),
                "wq": Wq[:, :, hs : hs + NH].transpose(0, 2, 1).reshape(DT, P, NW)
                .transpose(1, 0, 2).astype(NP_BF16),
                "wk": Wk[:, :, hs : hs + NH].transpose(0, 2, 1).reshape(DT, P, NW)
                .transpose(1, 0, 2).astype(NP_BF16),
                "wv": Wv[:, :, hs : hs + NH].transpose(0, 2, 1).reshape(DT, P, NW)
                .transpose(1, 0, 2).astype(NP_BF16),
                "wkern": wkern_pair.astype(NP_BF16),
                "masks": masks,
            }
        )
    return in_maps


def gather_output(results):
    out = np.zeros((B, S, KS), np.float32)
    for c in range(NCORES):
        b = c // CORES_PER_B
        oT = (
            np.asarray(results[c]["outT"], np.float32)  # [KS, NIB, NH, IB]
            .transpose(2, 0, 1, 3)
            .reshape(NH, KS, S)
        )
        zz = np.asarray(results[c]["z"], np.float32)     # [NH, S]
        out[b] += (oT / zz[:, None, :]).sum(axis=0).T
    return out


_NC_CACHE = {}


def get_nc():
    if "nc" not in _NC_CACHE:
        _NC_CACHE["nc"] = build_nc()
    return _NC_CACHE["nc"]


def run_hw(inputs, trace=False, **kw):
    from concourse.bass_utils import run_bass_kernel_spmd

    nc = get_nc()
    in_maps = make_in_maps(inputs)
    res = run_bass_kernel_spmd(
        nc, in_maps, list(range(NCORES)), trace=trace, **kw
    )
    return gather_output(res.results), res


def kernel(**inputs) -> np.ndarray:
    out, _ = run_hw(inputs, trace=False)
    return out
